# revision 25
# baseline (speedup 1.0000x reference)
"""Trainium2 Bass kernel for nn_GAT_15547781612261.

3-layer GATConv (6 heads, concat=False) over an 8192-node graph with self
loops, residual, returning final[ptr[1:]-1] -> [8, 1028].

Only the 8 output rows are needed, so compute is pruned to their 3-hop
in-neighborhood (L1: ~993 edges / 207 dst, L2: 250/50, L3: 50/8).  The 8
NeuronCores genuinely share the work:

  * L1 (the wide 1028-ch projection, the dominant FLOP+DMA cost) is sharded
    by destination node: the 207 L1-dst nodes are bin-packed onto 8 cores
    (<=128 edges, <=32 dst each).  Each core loads only its own edge-routed
    x columns and computes X2 for its shard.
  * The per-core X2 shards are exchanged with ONE AllGather collective (the
    cost model charges a flat ~15us per collective, so the design uses
    exactly one, dispatched as early as possible).
  * L2 is replicated (every core computes all 50 X3 rows from gathered X2).
  * L3 is sharded by output channel: each core holds a 1/8 column slice of
    W3 and emits out[:, c-slice]; the host concatenates slices (no second
    collective needed).

Precision: everything bulky is fp8-e4m3 (x routings, W matrices, source
routing matrices, the X2 collective payload); h tiles and alpha-scaled
routing are bf16; the softmax statistics chain is fp32.  Empirical
end-to-end rel-err ~1e-3 against the 2e-2 gate.

Latency structure: attention logits es[src(e)]+ed[dst(e)] are accumulated
directly in PSUM from TWO host-routed input copies (XE = x[src(e)], XD =
x[dst(e)]) against the folded stat weights [W@a_src | W@a_dst] - no
node-major gather, no self-edge permutation, no intermediate copies.  The
softmax denominator is produced node-major (z^T = Zdst^T @ ex), 1/z is
routed back to edges with one matmul, and per-head 1/z is folded into the
alpha-scaled routing matrix (za) so the head mean happens inside a single
PSUM accumulation.  The wide fp8 projections run in DoubleRow perf mode
(0.5 cycles/row, host-paired K-tile operands).  Leaky-relu reads PSUM on
the vector engine (one PSUM operand per instruction); only Exp and copies
share the Activation queue so its function table never reloads.  L2/L3
constant transfers ride the collective window on the shared DMA pipe, and
a short warmup stream ramps the PE out of its slow pstate.

Host does integer-only graph slicing/routing plus input-independent weight
folding [W | W@a_src | W@a_dst]; every input-dependent FLOP runs on device.
"""

import numpy as np
import ml_dtypes

P = 128
H = 6
CORES = 8
BF = ml_dtypes.bfloat16
F8 = ml_dtypes.float8_e4m3

# test harness hooks
TRACE = False
LAST_RESULT = None

N_WARM = 4   # PE pstate warmup matmuls


def _pad(n, m=P):
    return ((n + m - 1) // m) * m


# ----------------------------------------------------------------------------
# host-side graph slicing (integer work only)
# ----------------------------------------------------------------------------

class _Pack:
    """Stacks [rows<=128*k, C] fp32 arrays into one [128, N] image (cast to
    np_dt) loaded with few DMAs; records per-block column offsets."""

    def __init__(self, name, np_dt):
        self.name = name
        self.np_dt = np_dt
        self.cols = 0
        self.blocks = {}     # key -> (offset, block_cols, n_tiles)
        self.chunks = []

    def add(self, key, arr):
        arr = np.asarray(arr, np.float32)
        r, c = arr.shape
        if r <= P:
            tiles = [np.vstack([arr, np.zeros((P - r, c), np.float32)])
                     if r < P else arr]
        else:
            assert r % P == 0
            tiles = [arr[i * P:(i + 1) * P] for i in range(r // P)]
        self.blocks[key] = (self.cols, c, len(tiles))
        for t in tiles:
            self.chunks.append(np.ascontiguousarray(t.astype(self.np_dt)))
            self.cols += c

    def image(self):
        return np.ascontiguousarray(np.concatenate(self.chunks, axis=1))


def _fold_weights(W, a_src, a_dst):
    """[W | W_k @ as_k | W_k @ ad_k]  (input-independent host fold)."""
    W = np.asarray(W, np.float32)
    a_src = np.asarray(a_src, np.float32)
    a_dst = np.asarray(a_dst, np.float32)
    Cin = W.shape[0]
    C = a_src.shape[1]
    Wh = W.reshape(Cin, H, C)
    Was = np.einsum('ihc,hc->ih', Wh, a_src)
    Wad = np.einsum('ihc,hc->ih', Wh, a_dst)
    return W, Was, Wad


def _layer_edges(dst_set, src_all, dst_all):
    """Edges into dst_set, sorted by (dst, src); returns (src, dst) arrays."""
    mask = np.isin(dst_all, dst_set)
    s, d = src_all[mask], dst_all[mask]
    order = np.lexsort((s, d))
    return s[order], d[order]


def _host_prep(x, edge_index, ptr, params):
    x = np.ascontiguousarray(np.asarray(x, np.float32))
    N = x.shape[0]
    ei = np.asarray(edge_index, np.int64)
    ptr = np.asarray(ptr, np.int64)
    loops = np.arange(N, dtype=np.int64)
    src_all = np.concatenate([ei[0], loops])
    dst_all = np.concatenate([ei[1], loops])
    R = (ptr[1:] - 1) % N
    B = len(R)

    D3u = np.unique(R)
    s3, d3 = _layer_edges(D3u, src_all, dst_all)
    S3 = np.unique(s3)
    s2, d2 = _layer_edges(S3, src_all, dst_all)
    S2 = np.unique(s2)
    s1, d1 = _layer_edges(S2, src_all, dst_all)

    dims = [x.shape[1]] + [np.asarray(params[f'as{i}']).shape[1]
                           for i in (1, 2, 3)]
    C1, C2, C3 = dims[1], dims[2], dims[3]
    nK1 = _pad(dims[0]) // P

    # ---- L1 partition: bin-pack the L1-dst nodes onto 8 cores
    deg = np.array([(d1 == n).sum() for n in S2])
    EpC, DpC = P, 32
    while True:
        order = np.argsort(-deg, kind='stable')
        bins_e = [0] * CORES
        bins_n = [0] * CORES
        core_of = {}
        ok = True
        for i in order:
            best = None
            for b in range(CORES):
                if bins_n[b] < DpC and bins_e[b] + deg[i] <= EpC:
                    if best is None or bins_e[b] < bins_e[best]:
                        best = b
            if best is None:
                ok = False
                break
            core_of[int(S2[i])] = best
            bins_e[best] += int(deg[i])
            bins_n[best] += 1
        if ok:
            break
        EpC *= 2
        DpC *= 2
    nE1 = EpC // P

    slot_of = {}
    for n in S2:  # sorted -> deterministic slots
        c = core_of[int(n)]
        slot_of[int(n)] = sum(1 for m in slot_of if core_of[m] == c)
    grow = {int(n): core_of[int(n)] * DpC + slot_of[int(n)] for n in S2}
    Grows = CORES * DpC          # gathered X2 rows
    nGt = Grows // P

    W1, W1as, W1ad = _fold_weights(params['W1'], params['as1'], params['ad1'])
    W2, W2as, W2ad = _fold_weights(params['W2'], params['as2'], params['ad2'])
    W3, W3as, W3ad = _fold_weights(params['W3'], params['as3'], params['ad3'])
    W1s = np.concatenate([W1as, W1ad], 1)          # [1028, 12]
    W1sp = np.zeros((nK1 * P, 2 * H), np.float32)
    W1sp[:W1.shape[0]] = W1s
    W1mp = np.zeros((nK1 * P, H * C1), np.float32)
    W1mp[:W1.shape[0]] = W1

    # ---- L2 (shared by all cores)
    E2 = len(s2)
    E2p = _pad(E2)
    nE2 = E2p // P
    D2 = len(S3)
    D2p = 64 if D2 <= 64 else _pad(D2)
    s3loc = {int(n): i for i, n in enumerate(S3)}
    e2dst = np.array([s3loc[int(d)] for d in d2])
    Gsrc2 = np.zeros((Grows, E2p), np.float32)
    Gsrc2[[grow[int(s)] for s in s2], np.arange(E2)] = 1.0
    Gdst2 = np.zeros((Grows, E2p), np.float32)
    Gdst2[[grow[int(d)] for d in d2], np.arange(E2)] = 1.0
    Zdst2 = np.zeros((E2p, D2p), np.float32)
    Zdst2[np.arange(E2), e2dst] = 1.0
    ZdstTu2 = np.zeros((D2p, E2p), np.float32)
    ZdstTu2[e2dst, np.arange(E2)] = 1.0

    # ---- L3 (c-sharded)
    E3 = len(s3)
    E3p = _pad(E3)
    nE3 = E3p // P
    nd3 = len(D3u)
    nd3p = max(B, nd3)
    d3loc = {int(n): i for i, n in enumerate(D3u)}
    e3dst = np.array([d3loc[int(d)] for d in d3])
    nK3 = _pad(C2) // P
    Gsrc3 = np.zeros((D2p, E3p), np.float32)
    Gsrc3[[s3loc[int(s)] for s in s3], np.arange(E3)] = 1.0
    Gdst3 = np.zeros((D2p, E3p), np.float32)
    Gdst3[[s3loc[int(d)] for d in d3], np.arange(E3)] = 1.0
    Zdst3 = np.zeros((E3p, nd3p), np.float32)
    Zdst3[np.arange(E3), e3dst] = 1.0
    ZdstTu3 = np.zeros((nd3p, E3p), np.float32)
    ZdstTu3[e3dst, np.arange(E3)] = 1.0
    Zagg3 = np.zeros((E3p, B), np.float32)
    for j in range(B):
        Zagg3[np.arange(E3)[d3 == R[j]], j] = 1.0

    csplit = np.array_split(np.arange(C3), CORES)
    w3w = ((max(len(s) for s in csplit) + 3) // 4) * 4
    w3_widths = [len(s) for s in csplit]

    W2sp = np.concatenate([W2as, W2ad], 1)         # [128, 12]
    W3sp = np.concatenate([W3as, W3ad], 1)         # [256, 12]

    meta = dict(dims=dims, nK1=nK1, EpC=EpC, DpC=DpC, nE1=nE1,
                Grows=Grows, nGt=nGt, E2p=E2p, nE2=nE2, D2p=D2p,
                E3p=E3p, nE3=nE3, nd3p=nd3p, nK3=nK3, w3w=w3w, B=B,
                w3_widths=w3_widths)

    # ---- per-core pack images
    consts = []
    packs = None
    for c in range(CORES):
        nodes_c = [int(n) for n in S2 if core_of[int(n)] == c]
        emask = np.isin(d1, nodes_c)
        es, ed = s1[emask], d1[emask]
        E1c = len(es)
        assert E1c <= EpC and len(nodes_c) <= DpC
        edloc = np.array([slot_of[int(d)] for d in ed], dtype=np.int64)
        order = np.argsort(edloc, kind='stable')
        es, ed, edloc = es[order], ed[order], edloc[order]

        XE1T = np.zeros((nK1 * P, EpC), np.float32)
        XD1T = np.zeros((nK1 * P, EpC), np.float32)
        if E1c:
            XE1T[:dims[0], :E1c] = x[es].T
            XD1T[:dims[0], :E1c] = x[ed].T
        Zdst1 = np.zeros((EpC, DpC), np.float32)
        Zdst1[np.arange(E1c), edloc] = 1.0
        ZdstTu1 = np.zeros((DpC, EpC), np.float32)
        ZdstTu1[edloc, np.arange(E1c)] = 1.0

        cs = csplit[c]
        W3m_c = np.zeros((nK3 * P, H * w3w), np.float32)
        for h in range(H):
            W3m_c[:C2, h * w3w:h * w3w + len(cs)] = W3[:, h * C3 + cs[0]:
                                                       h * C3 + cs[-1] + 1]
        B3_c = np.zeros((B, w3w), np.float32)
        XR_c = np.zeros((B, w3w), np.float32)
        B3_c[:, :len(cs)] = np.asarray(params['b3'], np.float32)[None, cs]
        XR_c[:, :len(cs)] = x[R][:, cs]

        g8 = _Pack("g8", F8)
        # XE/XD tiles contiguous so adjacent K-tile pairs form DoubleRow
        # lhsT operands; stat/msg weights are stored pre-paired likewise.
        g8.add("XE1T", XE1T)
        g8.add("XD1T", XD1T)
        nPr = nK1 // 2
        Was1 = W1sp[:, 0:H]
        Wad1 = W1sp[:, H:2 * H]
        for j in range(nPr):
            g8.add(f"WasP_{j}", np.concatenate(
                [Was1[(2 * j) * P:(2 * j + 1) * P],
                 Was1[(2 * j + 1) * P:(2 * j + 2) * P]], axis=1))
            g8.add(f"WadP_{j}", np.concatenate(
                [Wad1[(2 * j) * P:(2 * j + 1) * P],
                 Wad1[(2 * j + 1) * P:(2 * j + 2) * P]], axis=1))
        if nK1 % 2:
            g8.add("Was_o", Was1[(nK1 - 1) * P:nK1 * P])
            g8.add("Wad_o", Wad1[(nK1 - 1) * P:nK1 * P])
        for ci, (n0, n1) in enumerate(_nchunks(H * C1, 512)):
            for j in range(nPr):
                g8.add(f"W1m{ci}_{j}", np.concatenate(
                    [W1mp[(2 * j) * P:(2 * j + 1) * P, n0:n1],
                     W1mp[(2 * j + 1) * P:(2 * j + 2) * P, n0:n1]], axis=1))
            if nK1 % 2:
                g8.add(f"W1m{ci}_o", W1mp[(nK1 - 1) * P:nK1 * P, n0:n1])
        # ---- late (transfers ride the collective window)
        g8.add("W2s", W2sp)
        g8.add("W2m", W2)
        g8.add("Gsrc2", Gsrc2)
        g8.add("Gdst2", Gdst2)
        nPr3 = nK3 // 2
        Was3 = W3sp[:, 0:H]
        Wad3 = W3sp[:, H:2 * H]
        for j in range(nPr3):
            g8.add(f"Was3P_{j}", np.concatenate(
                [Was3[(2 * j) * P:(2 * j + 1) * P],
                 Was3[(2 * j + 1) * P:(2 * j + 2) * P]], axis=1))
            g8.add(f"Wad3P_{j}", np.concatenate(
                [Wad3[(2 * j) * P:(2 * j + 1) * P],
                 Wad3[(2 * j + 1) * P:(2 * j + 2) * P]], axis=1))
        if nK3 % 2:
            g8.add("Was3_o", Was3[(nK3 - 1) * P:nK3 * P])
            g8.add("Wad3_o", Wad3[(nK3 - 1) * P:nK3 * P])
        for ci, (n0, n1) in enumerate(_nchunks(H * w3w, 512)):
            for j in range(nPr3):
                g8.add(f"W3m{ci}_{j}", np.concatenate(
                    [W3m_c[(2 * j) * P:(2 * j + 1) * P, n0:n1],
                     W3m_c[(2 * j + 1) * P:(2 * j + 2) * P, n0:n1]], axis=1))
            if nK3 % 2:
                g8.add(f"W3m{ci}_o", W3m_c[(nK3 - 1) * P:nK3 * P, n0:n1])
        g8.add("Gsrc3", Gsrc3)
        g8.add("Gdst3", Gdst3)

        gB = _Pack("gB", BF)
        gB.add("Zdst1b", Zdst1)
        gB.add("Zdst2b", Zdst2)        # late from here on
        gB.add("Zagg3b", Zagg3)

        gF = _Pack("gF", np.float32)
        gF.add("Zdst1f", Zdst1)
        gF.add("ZdstTu1", ZdstTu1)
        gF.add("B1", np.broadcast_to(
            np.asarray(params['b1'], np.float32)[None, :], (DpC, C1)).copy())
        gF.add("Zdst2f", Zdst2)        # late from here on
        gF.add("ZdstTu2", ZdstTu2)
        gF.add("B2", np.broadcast_to(
            np.asarray(params['b2'], np.float32)[None, :], (D2p, C2)).copy())
        gF.add("Zdst3f", Zdst3)
        gF.add("ZdstTu3", ZdstTu3)
        gF.add("B3", B3_c)
        gF.add("XR", XR_c)

        consts.append({"g8": g8.image(), "gB": gB.image(), "gF": gF.image()})
        if packs is None:
            packs = {"g8": g8, "gB": gB, "gF": gF}

    return consts, packs, meta, dims


# ----------------------------------------------------------------------------
# device program (identical on all cores; per-core behavior is in the data)
# ----------------------------------------------------------------------------

def _nchunks(total, step):
    out = []
    o = 0
    while o < total:
        out.append((o, min(o + step, total)))
        o += step
    return out


def _build_program(packs, meta, dims):
    import concourse.bacc as bacc
    import concourse.tile as tile
    from concourse import mybir
    from concourse.masks import make_identity

    f32 = mybir.dt.float32
    bf16 = mybir.dt.bfloat16
    fp8 = mybir.dt.float8e4
    Alu = mybir.AluOpType
    Act = mybir.ActivationFunctionType

    C1, C2, C3 = dims[1], dims[2], dims[3]
    nK1 = meta['nK1']
    EpC, DpC, nE1 = meta['EpC'], meta['DpC'], meta['nE1']
    Grows, nGt = meta['Grows'], meta['nGt']
    E2p, nE2, D2p = meta['E2p'], meta['nE2'], meta['D2p']
    E3p, nE3, nd3p, nK3 = meta['E3p'], meta['nE3'], meta['nd3p'], meta['nK3']
    w3w, B = meta['w3w'], meta['B']
    assert nE1 == 1 and nE3 == 1, "single-tile L1 shard / L3 edges expected"

    nc = bacc.Bacc("TRN2", target_bir_lowering=False)

    din = {
        "g8": nc.dram_tensor("g8", [P, packs["g8"].cols], fp8,
                             kind="ExternalInput"),
        "gB": nc.dram_tensor("gB", [P, packs["gB"].cols], bf16,
                             kind="ExternalInput"),
        "gF": nc.dram_tensor("gF", [P, packs["gF"].cols], f32,
                             kind="ExternalInput"),
    }
    dout = nc.dram_tensor("out", [B, w3w], f32, kind="ExternalOutput")

    ptile = {}

    def pv(grp, key, t=0, c0=None, c1=None, r=P):
        off, c, _n = packs[grp].blocks[key]
        lo = off + t * c + (c0 or 0)
        hi = off + t * c + (c1 if c1 is not None else c)
        return ptile[grp][:r, lo:hi]

    with tile.TileContext(nc) as tc:
        with tc.tile_pool(name="sb", bufs=1) as sb, \
             tc.tile_pool(name="psum", bufs=1, space="PSUM") as psum, \
             tc.tile_pool(name="dram", bufs=1, space="DRAM") as dram:
            ident = sb.tile([P, P], f32, name="ident", tag="ident")
            make_identity(nc, ident[:])

            for nm, dt in (("g8", fp8), ("gB", bf16), ("gF", f32)):
                ptile[nm] = sb.tile([P, packs[nm].cols], dt, name=f"pk_{nm}",
                                    tag=f"pk_{nm}")

            b8, bB, bF = (packs[n].blocks for n in ("g8", "gB", "gF"))
            w1m0 = b8["W1m0_0"][0]
            g8late = b8["W2s"][0]
            w1m_step = (g8late - w1m0) // 3
            emits = [("g8", 0, w1m0),
                     ("gF", 0, bF["B1"][0])]
            emits += [("g8", w1m0 + i * w1m_step, w1m0 + (i + 1) * w1m_step)
                      for i in range(3)]
            emits += [("gB", 0, bB["Zdst2b"][0]),
                      ("gF", bF["B1"][0], bF["Zdst2f"][0])]
            # late constants: emitted after the collective dispatch; their
            # transfers ride the collective window on the shared DMA pipe
            emits_late = [("g8", g8late, packs["g8"].cols),
                          ("gB", bB["Zdst2b"][0], packs["gB"].cols),
                          ("gF", bF["Zdst2f"][0], packs["gF"].cols)]
            for nm, c0, c1 in emits:
                nc.sync.dma_start(out=ptile[nm][:, c0:c1],
                                  in_=din[nm][:, c0:c1])

            x2b_in = dram.tile([DpC, C1], fp8, name="x2b_in", tag="x2b_in")
            x2b_out = dram.tile([Grows, C1], fp8, name="x2b_out",
                                tag="x2b_out")

            RR = [nc.vector, nc.gpsimd]   # za engines

            # PE pstate warmup (results discarded)
            wps = psum.tile([P, P], f32, name="ps_warm", tag="ps_warm",
                            bufs=1)
            for i in range(N_WARM):
                nc.tensor.matmul(out=wps[:], lhsT=ident[:], rhs=ident[:],
                                 start=(i % 8 == 0),
                                 stop=(i % 8 == 7 or i == N_WARM - 1))

            # stat tail: exp -> zT -> 1/z -> route -> alpha  (fp32)
            def stat_tail(li, Ep, nd, pl, Zdstf, ZdstTu, slope):
                exs = sb.tile([Ep, H], f32, name=f"exs{li}", tag=f"exs{li}")
                sx = sb.tile([Ep, H], f32, name=f"sx{li}", tag=f"sx{li}")
                nc.vector.tensor_scalar_mul(out=sx[:], in0=pl[:],
                                            scalar1=float(slope))
                nc.vector.tensor_tensor(out=exs[:], in0=sx[:], in1=pl[:],
                                        op=Alu.max)
                nc.scalar.activation(out=exs[:], in_=exs[:], func=Act.Exp)
                zp = psum.tile([nd, H], f32, name=f"ps_z{li}", tag="ps_small",
                               bufs=2)
                nc.tensor.matmul(out=zp[:], lhsT=Zdstf, rhs=exs[:],
                                 start=True, stop=True)
                rzT = sb.tile([nd, H], f32, name=f"rzT{li}", tag=f"rzT{li}")
                nc.vector.tensor_scalar_max(out=rzT[:], in0=zp[:],
                                            scalar1=1e-30)
                nc.vector.reciprocal(out=rzT[:], in_=rzT[:])
                psg = psum.tile([Ep, H], f32, name=f"ps_rzg{li}",
                                tag="ps_small", bufs=2)
                nc.tensor.matmul(out=psg[:], lhsT=ZdstTu, rhs=rzT[:],
                                 start=True, stop=True)
                al = sb.tile([Ep, H], f32, name=f"al{li}", tag=f"al{li}")
                nc.vector.tensor_tensor(out=al[:], in0=exs[:], in1=psg[:],
                                        op=Alu.mult)
                return al

            # ================= layer 1 (this core's dst shard) =============
            # logits straight into PSUM: es (XE x Was) + ed (XD x Wad)
            pl1 = psum.tile([EpC, H], f32, name="ps_lg1", tag="ps_small",
                            bufs=2)
            nPr = nK1 // 2
            DR = mybir.MatmulPerfMode.DoubleRow
            for j in range(nPr):
                nc.tensor.matmul(out=pl1[:],
                                 lhsT=pv("g8", "XE1T", 2 * j, 0, 2 * EpC),
                                 rhs=pv("g8", f"WasP_{j}"), perf_mode=DR,
                                 start=(j == 0), stop=False)
                nc.tensor.matmul(out=pl1[:],
                                 lhsT=pv("g8", "XD1T", 2 * j, 0, 2 * EpC),
                                 rhs=pv("g8", f"WadP_{j}"), perf_mode=DR,
                                 start=False, stop=False)
            if nK1 % 2:
                nc.tensor.matmul(out=pl1[:],
                                 lhsT=pv("g8", "XE1T", nK1 - 1),
                                 rhs=pv("g8", "Was_o"), start=False,
                                 stop=False)
                nc.tensor.matmul(out=pl1[:],
                                 lhsT=pv("g8", "XD1T", nK1 - 1),
                                 rhs=pv("g8", "Wad_o"), start=False,
                                 stop=True)
            al1 = stat_tail(1, EpC, DpC, pl1,
                            pv("gF", "Zdst1f", r=EpC),
                            pv("gF", "ZdstTu1", r=DpC), 0.2)

            # message projection [EpC, H*C1] in psum chunks -> bf16 sbuf
            h1 = sb.tile([EpC, H * C1], bf16, name="h1", tag="h1")
            for ci, (n0, n1) in enumerate(_nchunks(H * C1, 512)):
                ps = psum.tile([EpC, n1 - n0], f32, name=f"ps_m1_{ci}",
                               tag="ps_big", bufs=3)
                for j in range(nPr):
                    nc.tensor.matmul(out=ps[:],
                                     lhsT=pv("g8", "XE1T", 2 * j, 0,
                                             2 * EpC),
                                     rhs=pv("g8", f"W1m{ci}_{j}"),
                                     perf_mode=DR,
                                     start=(j == 0), stop=False)
                nc.tensor.matmul(out=ps[:],
                                 lhsT=pv("g8", "XE1T", nK1 - 1),
                                 rhs=pv("g8", f"W1m{ci}_o"),
                                 start=False, stop=True)
                # split the copy across both psum-capable engines
                half = (n1 - n0) // 2
                nc.vector.tensor_copy(out=h1[:, n0:n0 + half],
                                      in_=ps[:, 0:half])
                nc.scalar.copy(out=h1[:, n0 + half:n1],
                               in_=ps[:, half:n1 - n0])

            # za trick: psum-accumulated per-head aggregation (head mean free)
            pa1 = psum.tile([DpC, C1], f32, name="ps_x2", tag="ps_agg",
                            bufs=1)
            for h in range(H):
                za = sb.tile([EpC, DpC], bf16, name=f"za1_{h}",
                             tag=f"za1_{h}")
                nc.gpsimd.tensor_scalar_mul(out=za[:],
                                            in0=pv("gB", "Zdst1b", r=EpC),
                                            scalar1=al1[:, h:h + 1])
                nc.tensor.matmul(out=pa1[:], lhsT=za[:],
                                 rhs=h1[:, h * C1:(h + 1) * C1],
                                 start=(h == 0), stop=(h == H - 1))
            x2sb = sb.tile([DpC, C1], fp8, name="x2sb", tag="x2sb")
            nc.vector.scalar_tensor_tensor(
                out=x2sb[:], in0=pa1[:], scalar=1.0 / H,
                in1=pv("gF", "B1", r=DpC), op0=Alu.mult, op1=Alu.add)

            # ================= X2 all-gather ===============================
            nc.sync.dma_start(out=x2b_in[:], in_=x2sb[:])
            nc.gpsimd.collective_compute(
                "AllGather", Alu.bypass,
                replica_groups=[list(range(CORES))],
                ins=[x2b_in[:].opt()], outs=[x2b_out[:].opt()])
            for nm, c0, c1 in emits_late:
                nc.sync.dma_start(out=ptile[nm][:, c0:c1],
                                  in_=din[nm][:, c0:c1])
            X2all = sb.tile([P, nGt * C1], fp8, name="X2all", tag="X2all")
            nc.sync.dma_start(
                out=X2all[:].rearrange("p (t c) -> p t c", t=nGt),
                in_=x2b_out[:].rearrange("(t p) c -> p t c", t=nGt))
            X2 = [X2all[:, t * C1:(t + 1) * C1] for t in range(nGt)]

            # ================= layer 2 (replicated) ========================
            # src- and dst-routed edge-major X2: xe2 / xd2 [C1, E2p]
            xe2 = sb.tile([C1, E2p], fp8, name="xe2", tag="xe2")
            xd2 = sb.tile([C1, E2p], fp8, name="xd2", tag="xd2")
            psx = psum.tile([C1, E2p], f32, name="ps_xe2", tag="ps_big",
                            bufs=3)
            for t in range(nGt):
                nc.tensor.matmul(out=psx[:], lhsT=X2[t],
                                 rhs=pv("g8", "Gsrc2", t),
                                 start=(t == 0), stop=(t == nGt - 1))
            nc.vector.tensor_copy(out=xe2[:], in_=psx[:])
            psd = psum.tile([C1, E2p], f32, name="ps_xd2", tag="ps_big",
                            bufs=3)
            for t in range(nGt):
                nc.tensor.matmul(out=psd[:], lhsT=X2[t],
                                 rhs=pv("g8", "Gdst2", t),
                                 start=(t == 0), stop=(t == nGt - 1))
            nc.scalar.copy(out=xd2[:], in_=psd[:])

            # logits per edge tile straight into PSUM
            pl2 = psum.tile([P, nE2 * H], f32, name="ps_lg2", tag="ps_small",
                            bufs=2)
            for e in range(nE2):
                sl = pl2[:, e * H:(e + 1) * H]
                nc.tensor.matmul(out=sl, lhsT=xe2[:, e * P:(e + 1) * P],
                                 rhs=pv("g8", "W2s", 0, 0, H),
                                 start=True, stop=False)
                nc.tensor.matmul(out=sl, lhsT=xd2[:, e * P:(e + 1) * P],
                                 rhs=pv("g8", "W2s", 0, H, 2 * H),
                                 start=False, stop=True)
            exs2 = sb.tile([P, nE2 * H], f32, name="exs2", tag="exs2")
            sx2 = sb.tile([P, nE2 * H], f32, name="sx2", tag="sx2")
            nc.vector.tensor_scalar_mul(out=sx2[:], in0=pl2[:], scalar1=0.2)
            nc.vector.tensor_tensor(out=exs2[:], in0=sx2[:], in1=pl2[:],
                                    op=Alu.max)
            nc.scalar.activation(out=exs2[:], in_=exs2[:], func=Act.Exp)
            zp2 = psum.tile([D2p, H], f32, name="ps_z2", tag="ps_small",
                            bufs=2)
            for e in range(nE2):
                nc.tensor.matmul(out=zp2[:], lhsT=pv("gF", "Zdst2f", e, r=P),
                                 rhs=exs2[:, e * H:(e + 1) * H],
                                 start=(e == 0), stop=(e == nE2 - 1))
            rzT2 = sb.tile([D2p, H], f32, name="rzT2", tag="rzT2")
            nc.vector.tensor_scalar_max(out=rzT2[:], in0=zp2[:],
                                        scalar1=1e-30)
            nc.vector.reciprocal(out=rzT2[:], in_=rzT2[:])
            al2 = sb.tile([P, nE2 * H], f32, name="al2", tag="al2")
            for e in range(nE2):
                psg = psum.tile([P, H], f32, name=f"ps_rzg2{e}",
                                tag="ps_small", bufs=2)
                nc.tensor.matmul(out=psg[:],
                                 lhsT=pv("gF", "ZdstTu2", 0, e * P,
                                         (e + 1) * P, r=D2p),
                                 rhs=rzT2[:], start=True, stop=True)
                nc.vector.tensor_tensor(out=al2[:, e * H:(e + 1) * H],
                                        in0=exs2[:, e * H:(e + 1) * H],
                                        in1=psg[:], op=Alu.mult)

            # message projection per edge tile -> h2 bf16
            h2 = [sb.tile([P, H * C2], bf16, name=f"h2_{e}", tag=f"h2_{e}")
                  for e in range(nE2)]
            ci = 0
            for e in range(nE2):
                for (n0, n1) in _nchunks(H * C2, 512):
                    ps = psum.tile([P, n1 - n0], f32, name=f"ps_m2_{ci}",
                                   tag="ps_big", bufs=3)
                    nc.tensor.matmul(out=ps[:],
                                     lhsT=xe2[:, e * P:(e + 1) * P],
                                     rhs=pv("g8", "W2m", 0, n0, n1),
                                     start=True, stop=True)
                    if ci % 2 == 1:
                        nc.scalar.copy(out=h2[e][:, n0:n1], in_=ps[:])
                    else:
                        nc.vector.tensor_copy(out=h2[e][:, n0:n1], in_=ps[:])
                    ci += 1

            # aggregation: psum-accumulated matmuls (h, e)
            pa2 = psum.tile([D2p, C2], f32, name="ps_x3", tag="ps_agg",
                            bufs=1)
            first = True
            for h in range(H):
                for e in range(nE2):
                    za = sb.tile([P, D2p], bf16, name=f"za2_{h}_{e}",
                                 tag=f"za2_{h}_{e}")
                    eng = nc.gpsimd if (h % 3 == 2) else nc.vector
                    eng.tensor_scalar_mul(
                        out=za[:], in0=pv("gB", "Zdst2b", e, r=P),
                        scalar1=al2[:, e * H + h:e * H + h + 1])
                    nc.tensor.matmul(out=pa2[:], lhsT=za[:],
                                     rhs=h2[e][:, h * C2:(h + 1) * C2],
                                     start=first,
                                     stop=(h == H - 1 and e == nE2 - 1))
                    first = False
            x3sb = sb.tile([D2p, C2], fp8, name="x3sb", tag="x3sb")
            nc.vector.scalar_tensor_tensor(
                out=x3sb[:], in0=pa2[:], scalar=1.0 / H,
                in1=pv("gF", "B2", r=D2p), op0=Alu.mult, op1=Alu.add)

            # ================= layer 3 (column shard) ======================
            xe3 = sb.tile([P, nK3 * E3p], fp8, name="xe3", tag="xe3")
            xd3 = sb.tile([P, nK3 * E3p], fp8, name="xd3", tag="xd3")
            for m in range(nK3):
                psx3 = psum.tile([P, E3p], f32, name=f"ps_xe3{m}",
                                 tag="ps_small", bufs=2)
                nc.tensor.matmul(out=psx3[:],
                                 lhsT=x3sb[:, m * P:(m + 1) * P],
                                 rhs=pv("g8", "Gsrc3", r=D2p),
                                 start=True, stop=True)
                nc.vector.tensor_copy(out=xe3[:, m * E3p:(m + 1) * E3p],
                                      in_=psx3[:])
                psd3 = psum.tile([P, E3p], f32, name=f"ps_xd3{m}",
                                 tag="ps_small", bufs=2)
                nc.tensor.matmul(out=psd3[:],
                                 lhsT=x3sb[:, m * P:(m + 1) * P],
                                 rhs=pv("g8", "Gdst3", r=D2p),
                                 start=True, stop=True)
                nc.scalar.copy(out=xd3[:, m * E3p:(m + 1) * E3p],
                               in_=psd3[:])

            pl3 = psum.tile([E3p, H], f32, name="ps_lg3", tag="ps_small",
                            bufs=2)
            nPr3 = nK3 // 2
            for j in range(nPr3):
                nc.tensor.matmul(out=pl3[:],
                                 lhsT=xe3[:, 2 * j * E3p:(2 * j + 2) * E3p],
                                 rhs=pv("g8", f"Was3P_{j}"), perf_mode=DR,
                                 start=(j == 0), stop=False)
                nc.tensor.matmul(out=pl3[:],
                                 lhsT=xd3[:, 2 * j * E3p:(2 * j + 2) * E3p],
                                 rhs=pv("g8", f"Wad3P_{j}"), perf_mode=DR,
                                 start=False, stop=(nK3 % 2 == 0 and
                                                    j == nPr3 - 1))
            if nK3 % 2:
                nc.tensor.matmul(out=pl3[:],
                                 lhsT=xe3[:, (nK3 - 1) * E3p:nK3 * E3p],
                                 rhs=pv("g8", "Was3_o"), start=False,
                                 stop=False)
                nc.tensor.matmul(out=pl3[:],
                                 lhsT=xd3[:, (nK3 - 1) * E3p:nK3 * E3p],
                                 rhs=pv("g8", "Wad3_o"), start=False,
                                 stop=True)
            al3 = stat_tail(3, E3p, nd3p, pl3,
                            pv("gF", "Zdst3f", r=E3p),
                            pv("gF", "ZdstTu3", r=nd3p), 0.0)

            h3 = sb.tile([E3p, H * w3w], bf16, name="h3", tag="h3")
            ci = 0
            for (n0, n1) in _nchunks(H * w3w, 512):
                ps = psum.tile([E3p, n1 - n0], f32, name=f"ps_m3_{ci}",
                               tag="ps_big", bufs=3)
                for j in range(nPr3):
                    nc.tensor.matmul(out=ps[:],
                                     lhsT=xe3[:, 2 * j * E3p:
                                              (2 * j + 2) * E3p],
                                     rhs=pv("g8", f"W3m{ci}_{j}"),
                                     perf_mode=DR, start=(j == 0),
                                     stop=(nK3 % 2 == 0 and j == nPr3 - 1))
                if nK3 % 2:
                    nc.tensor.matmul(out=ps[:],
                                     lhsT=xe3[:, (nK3 - 1) * E3p:nK3 * E3p],
                                     rhs=pv("g8", f"W3m{ci}_o"),
                                     start=False, stop=True)
                if ci % 2 == 0:
                    nc.scalar.copy(out=h3[:, n0:n1], in_=ps[:])
                else:
                    nc.vector.tensor_copy(out=h3[:, n0:n1], in_=ps[:])
                ci += 1

            # residual + bias staged early (overlaps the collective)
            bxr = sb.tile([B, w3w], f32, name="bxr", tag="bxr")
            nc.vector.tensor_tensor(out=bxr[:], in0=pv("gF", "B3", r=B),
                                    in1=pv("gF", "XR", r=B), op=Alu.add)

            # final aggregation over the 8 output rows (Zagg alpha-scaled)
            pa3 = psum.tile([B, w3w], f32, name="ps_out", tag="ps_agg",
                            bufs=1)
            for h in range(H):
                za = sb.tile([E3p, B], bf16, name=f"za3_{h}", tag=f"za3_{h}")
                RR[h % 2].tensor_scalar_mul(out=za[:],
                                            in0=pv("gB", "Zagg3b", r=E3p),
                                            scalar1=al3[:, h:h + 1])
                nc.tensor.matmul(out=pa3[:], lhsT=za[:],
                                 rhs=h3[:, h * w3w:(h + 1) * w3w],
                                 start=(h == 0), stop=(h == H - 1))
            out_f = sb.tile([B, w3w], f32, name="out_f", tag="out_f")
            nc.vector.scalar_tensor_tensor(
                out=out_f[:], in0=pa3[:], scalar=1.0 / H, in1=bxr[:],
                op0=Alu.mult, op1=Alu.add)
            nc.sync.dma_start(out=dout[:], in_=out_f[:])

    nc.finalize()
    return nc


def kernel(**inputs):
    global LAST_RESULT
    consts, packs, meta, dims = _host_prep(
        inputs["x"], inputs["edge_index"], inputs["ptr"], inputs)
    nc = _build_program(packs, meta, dims)

    from concourse.bass_utils import run_bass_kernel_spmd
    res = run_bass_kernel_spmd(nc, consts, list(range(CORES)), trace=TRACE)
    LAST_RESULT = res
    cols = []
    for c in range(CORES):
        w = meta['w3_widths'][c]
        cols.append(np.asarray(res.results[c]["out"], np.float32)[:, :w])
    return np.concatenate(cols, axis=1)


# revision 28
# speedup vs baseline: 1.0043x; 1.0043x over previous
"""Trainium2 Bass kernel for nn_GAT_15547781612261.

3-layer GATConv (6 heads, concat=False) over an 8192-node graph with self
loops, residual, returning final[ptr[1:]-1] -> [8, 1028].

Only the 8 output rows are needed, so compute is pruned to their 3-hop
in-neighborhood (L1: ~993 edges / 207 dst, L2: 250/50, L3: 50/8).  The 8
NeuronCores genuinely share the work:

  * L1 (the wide 1028-ch projection, the dominant FLOP+DMA cost) is sharded
    by destination node: the 207 L1-dst nodes are bin-packed onto 8 cores
    (<=128 edges, <=32 dst each).  Each core loads only its own edge-routed
    x columns and computes X2 for its shard.
  * The per-core X2 shards are exchanged with ONE AllGather collective (the
    cost model charges a flat ~15us per collective, so the design uses
    exactly one, dispatched as early as possible).
  * L2 is replicated (every core computes all 50 X3 rows from gathered X2).
  * L3 is sharded by output channel: each core holds a 1/8 column slice of
    W3 and emits out[:, c-slice]; the host concatenates slices (no second
    collective needed).

Precision: everything bulky is fp8-e4m3 (x routings, W matrices, source
routing matrices, the X2 collective payload); h tiles and alpha-scaled
routing are bf16; the softmax statistics chain is fp32.  Empirical
end-to-end rel-err ~1e-3 against the 2e-2 gate.

Latency structure: attention logits es[src(e)]+ed[dst(e)] are accumulated
directly in PSUM from TWO host-routed input copies (XE = x[src(e)], XD =
x[dst(e)]) against the folded stat weights [W@a_src | W@a_dst] - no
node-major gather, no self-edge permutation, no intermediate copies.  The
softmax denominator is produced node-major (z^T = Zdst^T @ ex), 1/z is
routed back to edges with one matmul, and per-head 1/z is folded into the
alpha-scaled routing matrix (za) so the head mean happens inside a single
PSUM accumulation.  The wide fp8 projections run in DoubleRow perf mode
(0.5 cycles/row, host-paired K-tile operands).  Leaky-relu reads PSUM on
the vector engine (one PSUM operand per instruction); only Exp and copies
share the Activation queue so its function table never reloads.  L2/L3
constant transfers ride the collective window on the shared DMA pipe, and
a short warmup stream ramps the PE out of its slow pstate.

Host does integer-only graph slicing/routing plus input-independent weight
folding [W | W@a_src | W@a_dst]; every input-dependent FLOP runs on device.
"""

import numpy as np
import ml_dtypes

P = 128
H = 6
CORES = 8
BF = ml_dtypes.bfloat16
F8 = ml_dtypes.float8_e4m3

# test harness hooks
TRACE = False
LAST_RESULT = None

N_WARM = 4   # PE pstate warmup matmuls


def _pad(n, m=P):
    return ((n + m - 1) // m) * m


# ----------------------------------------------------------------------------
# host-side graph slicing (integer work only)
# ----------------------------------------------------------------------------

class _Pack:
    """Stacks [rows<=128*k, C] fp32 arrays into one [128, N] image (cast to
    np_dt) loaded with few DMAs; records per-block column offsets."""

    def __init__(self, name, np_dt):
        self.name = name
        self.np_dt = np_dt
        self.cols = 0
        self.blocks = {}     # key -> (offset, block_cols, n_tiles)
        self.chunks = []

    def add(self, key, arr):
        arr = np.asarray(arr, np.float32)
        r, c = arr.shape
        if r <= P:
            tiles = [np.vstack([arr, np.zeros((P - r, c), np.float32)])
                     if r < P else arr]
        else:
            assert r % P == 0
            tiles = [arr[i * P:(i + 1) * P] for i in range(r // P)]
        self.blocks[key] = (self.cols, c, len(tiles))
        for t in tiles:
            self.chunks.append(np.ascontiguousarray(t.astype(self.np_dt)))
            self.cols += c

    def image(self):
        return np.ascontiguousarray(np.concatenate(self.chunks, axis=1))


def _fold_weights(W, a_src, a_dst):
    """[W | W_k @ as_k | W_k @ ad_k]  (input-independent host fold)."""
    W = np.asarray(W, np.float32)
    a_src = np.asarray(a_src, np.float32)
    a_dst = np.asarray(a_dst, np.float32)
    Cin = W.shape[0]
    C = a_src.shape[1]
    Wh = W.reshape(Cin, H, C)
    Was = np.einsum('ihc,hc->ih', Wh, a_src)
    Wad = np.einsum('ihc,hc->ih', Wh, a_dst)
    return W, Was, Wad


def _layer_edges(dst_set, src_all, dst_all):
    """Edges into dst_set, sorted by (dst, src); returns (src, dst) arrays."""
    mask = np.isin(dst_all, dst_set)
    s, d = src_all[mask], dst_all[mask]
    order = np.lexsort((s, d))
    return s[order], d[order]


def _host_prep(x, edge_index, ptr, params):
    x = np.ascontiguousarray(np.asarray(x, np.float32))
    N = x.shape[0]
    ei = np.asarray(edge_index, np.int64)
    ptr = np.asarray(ptr, np.int64)
    loops = np.arange(N, dtype=np.int64)
    src_all = np.concatenate([ei[0], loops])
    dst_all = np.concatenate([ei[1], loops])
    R = (ptr[1:] - 1) % N
    B = len(R)

    D3u = np.unique(R)
    s3, d3 = _layer_edges(D3u, src_all, dst_all)
    S3 = np.unique(s3)
    s2, d2 = _layer_edges(S3, src_all, dst_all)
    S2 = np.unique(s2)
    s1, d1 = _layer_edges(S2, src_all, dst_all)

    dims = [x.shape[1]] + [np.asarray(params[f'as{i}']).shape[1]
                           for i in (1, 2, 3)]
    C1, C2, C3 = dims[1], dims[2], dims[3]
    nK1 = _pad(dims[0]) // P

    # ---- L1 partition: bin-pack the L1-dst nodes onto 8 cores
    deg = np.array([(d1 == n).sum() for n in S2])
    EpC, DpC = P, 32
    while True:
        order = np.argsort(-deg, kind='stable')
        bins_e = [0] * CORES
        bins_n = [0] * CORES
        core_of = {}
        ok = True
        for i in order:
            best = None
            for b in range(CORES):
                if bins_n[b] < DpC and bins_e[b] + deg[i] <= EpC:
                    if best is None or bins_e[b] < bins_e[best]:
                        best = b
            if best is None:
                ok = False
                break
            core_of[int(S2[i])] = best
            bins_e[best] += int(deg[i])
            bins_n[best] += 1
        if ok:
            break
        EpC *= 2
        DpC *= 2
    nE1 = EpC // P

    slot_of = {}
    for n in S2:  # sorted -> deterministic slots
        c = core_of[int(n)]
        slot_of[int(n)] = sum(1 for m in slot_of if core_of[m] == c)
    grow = {int(n): core_of[int(n)] * DpC + slot_of[int(n)] for n in S2}
    Grows = CORES * DpC          # gathered X2 rows
    nGt = Grows // P

    W1, W1as, W1ad = _fold_weights(params['W1'], params['as1'], params['ad1'])
    W2, W2as, W2ad = _fold_weights(params['W2'], params['as2'], params['ad2'])
    W3, W3as, W3ad = _fold_weights(params['W3'], params['as3'], params['ad3'])
    W1s = np.concatenate([W1as, W1ad], 1)          # [1028, 12]
    W1sp = np.zeros((nK1 * P, 2 * H), np.float32)
    W1sp[:W1.shape[0]] = W1s
    W1mp = np.zeros((nK1 * P, H * C1), np.float32)
    W1mp[:W1.shape[0]] = W1

    # ---- L2 (shared by all cores)
    E2 = len(s2)
    E2p = _pad(E2)
    nE2 = E2p // P
    D2 = len(S3)
    D2p = 64 if D2 <= 64 else _pad(D2)
    s3loc = {int(n): i for i, n in enumerate(S3)}
    e2dst = np.array([s3loc[int(d)] for d in d2])
    Gsrc2 = np.zeros((Grows, E2p), np.float32)
    Gsrc2[[grow[int(s)] for s in s2], np.arange(E2)] = 1.0
    Gdst2 = np.zeros((Grows, E2p), np.float32)
    Gdst2[[grow[int(d)] for d in d2], np.arange(E2)] = 1.0
    Zdst2 = np.zeros((E2p, D2p), np.float32)
    Zdst2[np.arange(E2), e2dst] = 1.0
    ZdstTu2 = np.zeros((D2p, E2p), np.float32)
    ZdstTu2[e2dst, np.arange(E2)] = 1.0

    # ---- L3 (c-sharded)
    E3 = len(s3)
    E3p = _pad(E3)
    nE3 = E3p // P
    nd3 = len(D3u)
    nd3p = max(B, nd3)
    d3loc = {int(n): i for i, n in enumerate(D3u)}
    e3dst = np.array([d3loc[int(d)] for d in d3])
    nK3 = _pad(C2) // P
    Gsrc3 = np.zeros((D2p, E3p), np.float32)
    Gsrc3[[s3loc[int(s)] for s in s3], np.arange(E3)] = 1.0
    Gdst3 = np.zeros((D2p, E3p), np.float32)
    Gdst3[[s3loc[int(d)] for d in d3], np.arange(E3)] = 1.0
    Zdst3 = np.zeros((E3p, nd3p), np.float32)
    Zdst3[np.arange(E3), e3dst] = 1.0
    ZdstTu3 = np.zeros((nd3p, E3p), np.float32)
    ZdstTu3[e3dst, np.arange(E3)] = 1.0
    Zagg3 = np.zeros((E3p, B), np.float32)
    for j in range(B):
        Zagg3[np.arange(E3)[d3 == R[j]], j] = 1.0

    csplit = np.array_split(np.arange(C3), CORES)
    w3w = ((max(len(s) for s in csplit) + 3) // 4) * 4
    w3_widths = [len(s) for s in csplit]

    W2sp = np.concatenate([W2as, W2ad], 1)         # [128, 12]
    W3sp = np.concatenate([W3as, W3ad], 1)         # [256, 12]

    meta = dict(dims=dims, nK1=nK1, EpC=EpC, DpC=DpC, nE1=nE1,
                Grows=Grows, nGt=nGt, E2p=E2p, nE2=nE2, D2p=D2p,
                E3p=E3p, nE3=nE3, nd3p=nd3p, nK3=nK3, w3w=w3w, B=B,
                w3_widths=w3_widths)

    # ---- per-core pack images
    consts = []
    packs = None
    for c in range(CORES):
        nodes_c = [int(n) for n in S2 if core_of[int(n)] == c]
        emask = np.isin(d1, nodes_c)
        es, ed = s1[emask], d1[emask]
        E1c = len(es)
        assert E1c <= EpC and len(nodes_c) <= DpC
        edloc = np.array([slot_of[int(d)] for d in ed], dtype=np.int64)
        order = np.argsort(edloc, kind='stable')
        es, ed, edloc = es[order], ed[order], edloc[order]

        XE1T = np.zeros((nK1 * P, EpC), np.float32)
        XD1T = np.zeros((nK1 * P, EpC), np.float32)
        if E1c:
            XE1T[:dims[0], :E1c] = x[es].T
            XD1T[:dims[0], :E1c] = x[ed].T
        Zdst1 = np.zeros((EpC, DpC), np.float32)
        Zdst1[np.arange(E1c), edloc] = 1.0
        ZdstTu1 = np.zeros((DpC, EpC), np.float32)
        ZdstTu1[edloc, np.arange(E1c)] = 1.0

        cs = csplit[c]
        W3m_c = np.zeros((nK3 * P, H * w3w), np.float32)
        for h in range(H):
            W3m_c[:C2, h * w3w:h * w3w + len(cs)] = W3[:, h * C3 + cs[0]:
                                                       h * C3 + cs[-1] + 1]
        B3_c = np.zeros((B, w3w), np.float32)
        XR_c = np.zeros((B, w3w), np.float32)
        B3_c[:, :len(cs)] = np.asarray(params['b3'], np.float32)[None, cs]
        XR_c[:, :len(cs)] = x[R][:, cs]

        g8 = _Pack("g8", F8)
        # XE/XD tiles contiguous so adjacent K-tile pairs form DoubleRow
        # lhsT operands; stat/msg weights are stored pre-paired likewise.
        g8.add("XE1T", XE1T)
        g8.add("XD1T", XD1T)
        nPr = nK1 // 2
        Was1 = W1sp[:, 0:H]
        Wad1 = W1sp[:, H:2 * H]
        for j in range(nPr):
            g8.add(f"WasP_{j}", np.concatenate(
                [Was1[(2 * j) * P:(2 * j + 1) * P],
                 Was1[(2 * j + 1) * P:(2 * j + 2) * P]], axis=1))
            g8.add(f"WadP_{j}", np.concatenate(
                [Wad1[(2 * j) * P:(2 * j + 1) * P],
                 Wad1[(2 * j + 1) * P:(2 * j + 2) * P]], axis=1))
        if nK1 % 2:
            g8.add("Was_o", Was1[(nK1 - 1) * P:nK1 * P])
            g8.add("Wad_o", Wad1[(nK1 - 1) * P:nK1 * P])
        for ci, (n0, n1) in enumerate(_nchunks(H * C1, 512)):
            for j in range(nPr):
                g8.add(f"W1m{ci}_{j}", np.concatenate(
                    [W1mp[(2 * j) * P:(2 * j + 1) * P, n0:n1],
                     W1mp[(2 * j + 1) * P:(2 * j + 2) * P, n0:n1]], axis=1))
            if nK1 % 2:
                g8.add(f"W1m{ci}_o", W1mp[(nK1 - 1) * P:nK1 * P, n0:n1])
        # ---- late (transfers ride the collective window)
        g8.add("W2s", W2sp)
        g8.add("W2m", W2)
        g8.add("Gsrc2", Gsrc2)
        g8.add("Gdst2", Gdst2)
        nPr3 = nK3 // 2
        Was3 = W3sp[:, 0:H]
        Wad3 = W3sp[:, H:2 * H]
        for j in range(nPr3):
            g8.add(f"Was3P_{j}", np.concatenate(
                [Was3[(2 * j) * P:(2 * j + 1) * P],
                 Was3[(2 * j + 1) * P:(2 * j + 2) * P]], axis=1))
            g8.add(f"Wad3P_{j}", np.concatenate(
                [Wad3[(2 * j) * P:(2 * j + 1) * P],
                 Wad3[(2 * j + 1) * P:(2 * j + 2) * P]], axis=1))
        if nK3 % 2:
            g8.add("Was3_o", Was3[(nK3 - 1) * P:nK3 * P])
            g8.add("Wad3_o", Wad3[(nK3 - 1) * P:nK3 * P])
        for ci, (n0, n1) in enumerate(_nchunks(H * w3w, 512)):
            for j in range(nPr3):
                g8.add(f"W3m{ci}_{j}", np.concatenate(
                    [W3m_c[(2 * j) * P:(2 * j + 1) * P, n0:n1],
                     W3m_c[(2 * j + 1) * P:(2 * j + 2) * P, n0:n1]], axis=1))
            if nK3 % 2:
                g8.add(f"W3m{ci}_o", W3m_c[(nK3 - 1) * P:nK3 * P, n0:n1])
        g8.add("Gsrc3", Gsrc3)
        g8.add("Gdst3", Gdst3)
        g8.add("Zdst2e", Zdst2)

        gB = _Pack("gB", BF)
        gB.add("Zdst1b", Zdst1)
        gB.add("Zdst2b", Zdst2)        # late from here on
        gB.add("Zagg3b", Zagg3)

        gF = _Pack("gF", np.float32)
        gF.add("Zdst1f", Zdst1)
        gF.add("ZdstTu1", ZdstTu1)
        gF.add("B1", np.broadcast_to(
            np.asarray(params['b1'], np.float32)[None, :], (DpC, C1)).copy())
        gF.add("Zdst2f", Zdst2)        # late from here on
        gF.add("ZdstTu2", ZdstTu2)
        gF.add("B2", np.broadcast_to(
            np.asarray(params['b2'], np.float32)[None, :], (D2p, C2)).copy())
        gF.add("Zdst3f", Zdst3)
        gF.add("ZdstTu3", ZdstTu3)
        gF.add("B3", B3_c)
        gF.add("XR", XR_c)

        consts.append({"g8": g8.image(), "gB": gB.image(), "gF": gF.image()})
        if packs is None:
            packs = {"g8": g8, "gB": gB, "gF": gF}

    return consts, packs, meta, dims


# ----------------------------------------------------------------------------
# device program (identical on all cores; per-core behavior is in the data)
# ----------------------------------------------------------------------------

def _nchunks(total, step):
    out = []
    o = 0
    while o < total:
        out.append((o, min(o + step, total)))
        o += step
    return out


def _build_program(packs, meta, dims):
    import concourse.bacc as bacc
    import concourse.tile as tile
    from concourse import mybir
    from concourse.masks import make_identity

    f32 = mybir.dt.float32
    bf16 = mybir.dt.bfloat16
    fp8 = mybir.dt.float8e4
    Alu = mybir.AluOpType
    Act = mybir.ActivationFunctionType

    C1, C2, C3 = dims[1], dims[2], dims[3]
    nK1 = meta['nK1']
    EpC, DpC, nE1 = meta['EpC'], meta['DpC'], meta['nE1']
    Grows, nGt = meta['Grows'], meta['nGt']
    E2p, nE2, D2p = meta['E2p'], meta['nE2'], meta['D2p']
    E3p, nE3, nd3p, nK3 = meta['E3p'], meta['nE3'], meta['nd3p'], meta['nK3']
    w3w, B = meta['w3w'], meta['B']
    assert nE1 == 1 and nE3 == 1, "single-tile L1 shard / L3 edges expected"

    nc = bacc.Bacc("TRN2", target_bir_lowering=False)

    din = {
        "g8": nc.dram_tensor("g8", [P, packs["g8"].cols], fp8,
                             kind="ExternalInput"),
        "gB": nc.dram_tensor("gB", [P, packs["gB"].cols], bf16,
                             kind="ExternalInput"),
        "gF": nc.dram_tensor("gF", [P, packs["gF"].cols], f32,
                             kind="ExternalInput"),
    }
    dout = nc.dram_tensor("out", [B, w3w], f32, kind="ExternalOutput")

    ptile = {}

    def pv(grp, key, t=0, c0=None, c1=None, r=P):
        off, c, _n = packs[grp].blocks[key]
        lo = off + t * c + (c0 or 0)
        hi = off + t * c + (c1 if c1 is not None else c)
        return ptile[grp][:r, lo:hi]

    with tile.TileContext(nc) as tc:
        with tc.tile_pool(name="sb", bufs=1) as sb, \
             tc.tile_pool(name="psum", bufs=1, space="PSUM") as psum, \
             tc.tile_pool(name="dram", bufs=1, space="DRAM") as dram:
            ident = sb.tile([P, P], f32, name="ident", tag="ident")
            make_identity(nc, ident[:])

            for nm, dt in (("g8", fp8), ("gB", bf16), ("gF", f32)):
                ptile[nm] = sb.tile([P, packs[nm].cols], dt, name=f"pk_{nm}",
                                    tag=f"pk_{nm}")

            b8, bB, bF = (packs[n].blocks for n in ("g8", "gB", "gF"))
            w1m0 = b8["W1m0_0"][0]
            g8late = b8["W2s"][0]
            nwc = 3
            w1m_step = (g8late - w1m0) // nwc
            emits = [("g8", 0, w1m0),
                     ("gF", 0, bF["B1"][0])]
            emits += [("g8", w1m0 + i * w1m_step,
                       g8late if i == nwc - 1 else
                       w1m0 + (i + 1) * w1m_step)
                      for i in range(nwc)]
            emits += [("gB", 0, bB["Zdst2b"][0]),
                      ("gF", bF["B1"][0], bF["Zdst2f"][0])]
            # late constants: emitted after the collective dispatch; their
            # transfers ride the collective window on the shared DMA pipe
            emits_late = [("g8", g8late, packs["g8"].cols),
                          ("gB", bB["Zdst2b"][0], packs["gB"].cols),
                          ("gF", bF["Zdst2f"][0], packs["gF"].cols)]
            for nm, c0, c1 in emits:
                nc.sync.dma_start(out=ptile[nm][:, c0:c1],
                                  in_=din[nm][:, c0:c1])

            x2b_in = dram.tile([DpC, C1], fp8, name="x2b_in", tag="x2b_in")
            x2b_out = dram.tile([Grows, C1], fp8, name="x2b_out",
                                tag="x2b_out")

            RR = [nc.vector, nc.gpsimd]   # za engines

            # PE pstate warmup (results discarded)
            wps = psum.tile([P, P], f32, name="ps_warm", tag="ps_warm",
                            bufs=1)
            for i in range(N_WARM):
                nc.tensor.matmul(out=wps[:], lhsT=ident[:], rhs=ident[:],
                                 start=(i % 8 == 0),
                                 stop=(i % 8 == 7 or i == N_WARM - 1))

            # stat tail: exp -> zT -> 1/z -> route -> alpha  (fp32)
            def stat_tail(li, Ep, nd, pl, Zdstf, ZdstTu, slope):
                exs = sb.tile([Ep, H], f32, name=f"exs{li}", tag=f"exs{li}")
                sx = sb.tile([Ep, H], f32, name=f"sx{li}", tag=f"sx{li}")
                nc.vector.tensor_scalar_mul(out=sx[:], in0=pl[:],
                                            scalar1=float(slope))
                nc.vector.tensor_tensor(out=exs[:], in0=sx[:], in1=pl[:],
                                        op=Alu.max)
                nc.scalar.activation(out=exs[:], in_=exs[:], func=Act.Exp)
                zp = psum.tile([nd, H], f32, name=f"ps_z{li}", tag="ps_small",
                               bufs=2)
                nc.tensor.matmul(out=zp[:], lhsT=Zdstf, rhs=exs[:],
                                 start=True, stop=True)
                rzT = sb.tile([nd, H], f32, name=f"rzT{li}", tag=f"rzT{li}")
                nc.vector.tensor_scalar_max(out=rzT[:], in0=zp[:],
                                            scalar1=1e-30)
                nc.vector.reciprocal(out=rzT[:], in_=rzT[:])
                psg = psum.tile([Ep, H], f32, name=f"ps_rzg{li}",
                                tag="ps_small", bufs=2)
                nc.tensor.matmul(out=psg[:], lhsT=ZdstTu, rhs=rzT[:],
                                 start=True, stop=True)
                al = sb.tile([Ep, H], f32, name=f"al{li}", tag=f"al{li}")
                nc.vector.tensor_tensor(out=al[:], in0=exs[:], in1=psg[:],
                                        op=Alu.mult)
                return al

            # ================= layer 1 (this core's dst shard) =============
            # logits straight into PSUM: es (XE x Was) + ed (XD x Wad)
            pl1 = psum.tile([EpC, H], f32, name="ps_lg1", tag="ps_small",
                            bufs=2)
            nPr = nK1 // 2
            DR = mybir.MatmulPerfMode.DoubleRow
            for j in range(nPr):
                nc.tensor.matmul(out=pl1[:],
                                 lhsT=pv("g8", "XE1T", 2 * j, 0, 2 * EpC),
                                 rhs=pv("g8", f"WasP_{j}"), perf_mode=DR,
                                 start=(j == 0), stop=False)
                nc.tensor.matmul(out=pl1[:],
                                 lhsT=pv("g8", "XD1T", 2 * j, 0, 2 * EpC),
                                 rhs=pv("g8", f"WadP_{j}"), perf_mode=DR,
                                 start=False, stop=False)
            if nK1 % 2:
                nc.tensor.matmul(out=pl1[:],
                                 lhsT=pv("g8", "XE1T", nK1 - 1),
                                 rhs=pv("g8", "Was_o"), start=False,
                                 stop=False)
                nc.tensor.matmul(out=pl1[:],
                                 lhsT=pv("g8", "XD1T", nK1 - 1),
                                 rhs=pv("g8", "Wad_o"), start=False,
                                 stop=True)
            al1 = stat_tail(1, EpC, DpC, pl1,
                            pv("gF", "Zdst1f", r=EpC),
                            pv("gF", "ZdstTu1", r=DpC), 0.2)

            # message projection [EpC, H*C1] in psum chunks -> bf16 sbuf
            h1 = sb.tile([EpC, H * C1], bf16, name="h1", tag="h1")
            for ci, (n0, n1) in enumerate(_nchunks(H * C1, 512)):
                ps = psum.tile([EpC, n1 - n0], f32, name=f"ps_m1_{ci}",
                               tag="ps_big", bufs=3)
                for j in range(nPr):
                    nc.tensor.matmul(out=ps[:],
                                     lhsT=pv("g8", "XE1T", 2 * j, 0,
                                             2 * EpC),
                                     rhs=pv("g8", f"W1m{ci}_{j}"),
                                     perf_mode=DR,
                                     start=(j == 0), stop=False)
                nc.tensor.matmul(out=ps[:],
                                 lhsT=pv("g8", "XE1T", nK1 - 1),
                                 rhs=pv("g8", f"W1m{ci}_o"),
                                 start=False, stop=True)
                # split the copy across both psum-capable engines
                half = (n1 - n0) // 2
                nc.vector.tensor_copy(out=h1[:, n0:n0 + half],
                                      in_=ps[:, 0:half])
                nc.scalar.copy(out=h1[:, n0 + half:n1],
                               in_=ps[:, half:n1 - n0])

            # za trick: psum-accumulated per-head aggregation (head mean free)
            pa1 = psum.tile([DpC, C1], f32, name="ps_x2", tag="ps_agg",
                            bufs=1)
            for h in range(H):
                za = sb.tile([EpC, DpC], bf16, name=f"za1_{h}",
                             tag=f"za1_{h}")
                eng = nc.gpsimd if h % 2 else nc.vector
                eng.tensor_scalar_mul(out=za[:],
                                      in0=pv("gB", "Zdst1b", r=EpC),
                                      scalar1=al1[:, h:h + 1])
                nc.tensor.matmul(out=pa1[:], lhsT=za[:],
                                 rhs=h1[:, h * C1:(h + 1) * C1],
                                 start=(h == 0), stop=(h == H - 1))
            x2sb = sb.tile([DpC, C1], fp8, name="x2sb", tag="x2sb")
            nc.vector.scalar_tensor_tensor(
                out=x2sb[:], in0=pa1[:], scalar=1.0 / H,
                in1=pv("gF", "B1", r=DpC), op0=Alu.mult, op1=Alu.add)

            # ================= X2 all-gather ===============================
            nc.sync.dma_start(out=x2b_in[:], in_=x2sb[:])
            nc.gpsimd.collective_compute(
                "AllGather", Alu.bypass,
                replica_groups=[list(range(CORES))],
                ins=[x2b_in[:].opt()], outs=[x2b_out[:].opt()])
            for nm, c0, c1 in emits_late:
                nc.sync.dma_start(out=ptile[nm][:, c0:c1],
                                  in_=din[nm][:, c0:c1])
            X2all = sb.tile([P, nGt * C1], fp8, name="X2all", tag="X2all")
            nc.sync.dma_start(
                out=X2all[:].rearrange("p (t c) -> p t c", t=nGt),
                in_=x2b_out[:].rearrange("(t p) c -> p t c", t=nGt))
            X2 = [X2all[:, t * C1:(t + 1) * C1] for t in range(nGt)]

            # ================= layer 2 (replicated) ========================
            # src- and dst-routed edge-major X2: xe2 / xd2 [C1, E2p]
            xe2 = sb.tile([C1, E2p], fp8, name="xe2", tag="xe2")
            xd2 = sb.tile([C1, E2p], fp8, name="xd2", tag="xd2")
            psx = psum.tile([C1, E2p], f32, name="ps_xe2", tag="ps_big",
                            bufs=3)
            for t in range(nGt):
                nc.tensor.matmul(out=psx[:], lhsT=X2[t],
                                 rhs=pv("g8", "Gsrc2", t),
                                 start=(t == 0), stop=(t == nGt - 1))
            nc.vector.tensor_copy(out=xe2[:], in_=psx[:])
            psd = psum.tile([C1, E2p], f32, name="ps_xd2", tag="ps_big",
                            bufs=3)
            for t in range(nGt):
                nc.tensor.matmul(out=psd[:], lhsT=X2[t],
                                 rhs=pv("g8", "Gdst2", t),
                                 start=(t == 0), stop=(t == nGt - 1))
            nc.scalar.copy(out=xd2[:], in_=psd[:])

            # logits per edge tile straight into PSUM
            pl2 = psum.tile([P, nE2 * H], f32, name="ps_lg2", tag="ps_small",
                            bufs=2)
            for e in range(nE2):
                sl = pl2[:, e * H:(e + 1) * H]
                nc.tensor.matmul(out=sl, lhsT=xe2[:, e * P:(e + 1) * P],
                                 rhs=pv("g8", "W2s", 0, 0, H),
                                 start=True, stop=False)
                nc.tensor.matmul(out=sl, lhsT=xd2[:, e * P:(e + 1) * P],
                                 rhs=pv("g8", "W2s", 0, H, 2 * H),
                                 start=False, stop=True)
            exs2 = sb.tile([P, nE2 * H], f32, name="exs2", tag="exs2")
            sx2 = sb.tile([P, nE2 * H], f32, name="sx2", tag="sx2")
            nc.vector.tensor_scalar_mul(out=sx2[:], in0=pl2[:], scalar1=0.2)
            nc.vector.tensor_tensor(out=exs2[:], in0=sx2[:], in1=pl2[:],
                                    op=Alu.max)
            nc.scalar.activation(out=exs2[:], in_=exs2[:], func=Act.Exp)
            zp2 = psum.tile([D2p, H], f32, name="ps_z2", tag="ps_small",
                            bufs=2)
            for e in range(nE2):
                nc.tensor.matmul(out=zp2[:], lhsT=pv("gF", "Zdst2f", e, r=P),
                                 rhs=exs2[:, e * H:(e + 1) * H],
                                 start=(e == 0), stop=(e == nE2 - 1))
            rzT2 = sb.tile([D2p, H], f32, name="rzT2", tag="rzT2")
            nc.vector.tensor_scalar_max(out=rzT2[:], in0=zp2[:],
                                        scalar1=1e-30)
            nc.vector.reciprocal(out=rzT2[:], in_=rzT2[:])
            al2 = sb.tile([P, nE2 * H], f32, name="al2", tag="al2")
            for e in range(nE2):
                psg = psum.tile([P, H], f32, name=f"ps_rzg2{e}",
                                tag="ps_small", bufs=2)
                nc.tensor.matmul(out=psg[:],
                                 lhsT=pv("gF", "ZdstTu2", 0, e * P,
                                         (e + 1) * P, r=D2p),
                                 rhs=rzT2[:], start=True, stop=True)
                nc.vector.tensor_tensor(out=al2[:, e * H:(e + 1) * H],
                                        in0=exs2[:, e * H:(e + 1) * H],
                                        in1=psg[:], op=Alu.mult)

            # message projection; h2 stored fp8 with per-head (e0|e1) pairs
            # so aggregation runs in DoubleRow mode (6 matmuls, not 12)
            h2p = sb.tile([P, nE2 * H * C2], fp8, name="h2p", tag="h2p")
            ci = 0
            for e in range(nE2):
                for (n0, n1) in _nchunks(H * C2, 512):
                    ps = psum.tile([P, n1 - n0], f32, name=f"ps_m2_{ci}",
                                   tag="ps_big", bufs=3)
                    nc.tensor.matmul(out=ps[:],
                                     lhsT=xe2[:, e * P:(e + 1) * P],
                                     rhs=pv("g8", "W2m", 0, n0, n1),
                                     start=True, stop=True)
                    for hh in range(n0 // C2, n1 // C2):
                        dst = h2p[:, (hh * nE2 + e) * C2:
                                  (hh * nE2 + e + 1) * C2]
                        srcp = ps[:, hh * C2 - n0:(hh + 1) * C2 - n0]
                        if (ci + hh) % 2 == 1:
                            nc.scalar.copy(out=dst, in_=srcp)
                        else:
                            nc.vector.tensor_copy(out=dst, in_=srcp)
                    ci += 1

            # aggregation: fp8 DoubleRow pairs (e0|e1) per head
            pa2 = psum.tile([D2p, C2], f32, name="ps_x3", tag="ps_agg",
                            bufs=1)
            for h in range(H):
                za = sb.tile([P, nE2 * D2p], fp8, name=f"za2_{h}",
                             tag=f"za2_{h}")
                for e in range(nE2):
                    eng = nc.gpsimd if (h % 3 == 2) else nc.vector
                    eng.tensor_scalar_mul(
                        out=za[:, e * D2p:(e + 1) * D2p],
                        in0=pv("g8", "Zdst2e", e, r=P),
                        scalar1=al2[:, e * H + h:e * H + h + 1])
                nc.tensor.matmul(
                    out=pa2[:], lhsT=dr(za[:]),
                    rhs=dr(h2p[:, h * nE2 * C2:(h + 1) * nE2 * C2]),
                    perf_mode=DR, start=(h == 0), stop=(h == H - 1))
            x3sb = sb.tile([D2p, C2], fp8, name="x3sb", tag="x3sb")
            nc.vector.scalar_tensor_tensor(
                out=x3sb[:], in0=pa2[:], scalar=1.0 / H,
                in1=pv("gF", "B2", r=D2p), op0=Alu.mult, op1=Alu.add)

            # ================= layer 3 (column shard) ======================
            xe3 = sb.tile([P, nK3 * E3p], fp8, name="xe3", tag="xe3")
            xd3 = sb.tile([P, nK3 * E3p], fp8, name="xd3", tag="xd3")
            for m in range(nK3):
                psx3 = psum.tile([P, E3p], f32, name=f"ps_xe3{m}",
                                 tag="ps_small", bufs=2)
                nc.tensor.matmul(out=psx3[:],
                                 lhsT=x3sb[:, m * P:(m + 1) * P],
                                 rhs=pv("g8", "Gsrc3", r=D2p),
                                 start=True, stop=True)
                nc.vector.tensor_copy(out=xe3[:, m * E3p:(m + 1) * E3p],
                                      in_=psx3[:])
                psd3 = psum.tile([P, E3p], f32, name=f"ps_xd3{m}",
                                 tag="ps_small", bufs=2)
                nc.tensor.matmul(out=psd3[:],
                                 lhsT=x3sb[:, m * P:(m + 1) * P],
                                 rhs=pv("g8", "Gdst3", r=D2p),
                                 start=True, stop=True)
                nc.scalar.copy(out=xd3[:, m * E3p:(m + 1) * E3p],
                               in_=psd3[:])

            pl3 = psum.tile([E3p, H], f32, name="ps_lg3", tag="ps_small",
                            bufs=2)
            nPr3 = nK3 // 2
            for j in range(nPr3):
                nc.tensor.matmul(out=pl3[:],
                                 lhsT=xe3[:, 2 * j * E3p:(2 * j + 2) * E3p],
                                 rhs=pv("g8", f"Was3P_{j}"), perf_mode=DR,
                                 start=(j == 0), stop=False)
                nc.tensor.matmul(out=pl3[:],
                                 lhsT=xd3[:, 2 * j * E3p:(2 * j + 2) * E3p],
                                 rhs=pv("g8", f"Wad3P_{j}"), perf_mode=DR,
                                 start=False, stop=(nK3 % 2 == 0 and
                                                    j == nPr3 - 1))
            if nK3 % 2:
                nc.tensor.matmul(out=pl3[:],
                                 lhsT=xe3[:, (nK3 - 1) * E3p:nK3 * E3p],
                                 rhs=pv("g8", "Was3_o"), start=False,
                                 stop=False)
                nc.tensor.matmul(out=pl3[:],
                                 lhsT=xd3[:, (nK3 - 1) * E3p:nK3 * E3p],
                                 rhs=pv("g8", "Wad3_o"), start=False,
                                 stop=True)
            al3 = stat_tail(3, E3p, nd3p, pl3,
                            pv("gF", "Zdst3f", r=E3p),
                            pv("gF", "ZdstTu3", r=nd3p), 0.0)

            h3 = sb.tile([E3p, H * w3w], bf16, name="h3", tag="h3")
            ci = 0
            for (n0, n1) in _nchunks(H * w3w, 512):
                ps = psum.tile([E3p, n1 - n0], f32, name=f"ps_m3_{ci}",
                               tag="ps_big", bufs=3)
                for j in range(nPr3):
                    nc.tensor.matmul(out=ps[:],
                                     lhsT=xe3[:, 2 * j * E3p:
                                              (2 * j + 2) * E3p],
                                     rhs=pv("g8", f"W3m{ci}_{j}"),
                                     perf_mode=DR, start=(j == 0),
                                     stop=(nK3 % 2 == 0 and j == nPr3 - 1))
                if nK3 % 2:
                    nc.tensor.matmul(out=ps[:],
                                     lhsT=xe3[:, (nK3 - 1) * E3p:nK3 * E3p],
                                     rhs=pv("g8", f"W3m{ci}_o"),
                                     start=False, stop=True)
                if ci % 2 == 0:
                    nc.scalar.copy(out=h3[:, n0:n1], in_=ps[:])
                else:
                    nc.vector.tensor_copy(out=h3[:, n0:n1], in_=ps[:])
                ci += 1

            # residual + bias staged early (overlaps the collective)
            bxr = sb.tile([B, w3w], f32, name="bxr", tag="bxr")
            nc.vector.tensor_tensor(out=bxr[:], in0=pv("gF", "B3", r=B),
                                    in1=pv("gF", "XR", r=B), op=Alu.add)

            # final aggregation over the 8 output rows (Zagg alpha-scaled)
            pa3 = psum.tile([B, w3w], f32, name="ps_out", tag="ps_agg",
                            bufs=1)
            for h in range(H):
                za = sb.tile([E3p, B], bf16, name=f"za3_{h}", tag=f"za3_{h}")
                RR[h % 2].tensor_scalar_mul(out=za[:],
                                            in0=pv("gB", "Zagg3b", r=E3p),
                                            scalar1=al3[:, h:h + 1])
                nc.tensor.matmul(out=pa3[:], lhsT=za[:],
                                 rhs=h3[:, h * w3w:(h + 1) * w3w],
                                 start=(h == 0), stop=(h == H - 1))
            out_f = sb.tile([B, w3w], f32, name="out_f", tag="out_f")
            nc.vector.scalar_tensor_tensor(
                out=out_f[:], in0=pa3[:], scalar=1.0 / H, in1=bxr[:],
                op0=Alu.mult, op1=Alu.add)
            nc.sync.dma_start(out=dout[:], in_=out_f[:])

    nc.finalize()
    return nc


def kernel(**inputs):
    global LAST_RESULT
    consts, packs, meta, dims = _host_prep(
        inputs["x"], inputs["edge_index"], inputs["ptr"], inputs)
    nc = _build_program(packs, meta, dims)

    from concourse.bass_utils import run_bass_kernel_spmd
    res = run_bass_kernel_spmd(nc, consts, list(range(CORES)), trace=TRACE)
    LAST_RESULT = res
    cols = []
    for c in range(CORES):
        w = meta['w3_widths'][c]
        cols.append(np.asarray(res.results[c]["out"], np.float32)[:, :w])
    return np.concatenate(cols, axis=1)
